# revision 32
# baseline (speedup 1.0000x reference)
"""Trainium2 Bass kernel for nn_Attention_46901042872659.

Dense transformer attention block:
  qkv = BN(x @ qkv_w.T); split q,k,v per head; attn = softmax(q k^T * scale + bias);
  out = hardswish(attn @ v); y = BN(out @ proj_w.T)

Strategy: data-parallel over batch across 8 NeuronCores (8 batch elems each).
Fully fused per batch element: qkv GEMM -> attention -> proj all stay on-chip
(no DRAM scratch). qkv/proj GEMMs and attention matmuls run bf16 (full PE
rate); PSUM accumulation is f32. Eval-mode BN is folded into GEMM weights/bias
on the host; softmax scale folded into q weights; relative-position bias
gathered host-side into a dense [heads, j, i] matrix.

Per-core program (SPMD on 8 cores), per batch elem b (8 per core):
  - qk GEMM: out channel-major [co 128, t 256]; cot 0-5 = q blocks (heads
    2j/2j+1 in partition halves), cot 6-11 = k blocks. Bias rides ACT.
  - v GEMM: out token-major [t 128, co 3072]; bias rides the DVE mover.
  - per head: S psum preloaded with attn bias (ACT copy), S += k^T q;
    exp on ACT; rowsum via ones-matmul; 1/rowsum (DVE); partition-broadcast
    via rank-1 PE matmul staged to SBUF on ACT; O = v^T expS;
    hardswish via relu6 form: h = y0 * min(Relu(y0+3), 6)/6, y0 = O/rowsum.
  - proj GEMM interleaved one batch-elem behind; BN fold on DVE.
"""
import numpy as np
import ml_dtypes
from contextlib import ExitStack

import concourse.bass as bass
import concourse.tile as tile
from concourse import bacc, bass_isa, mybir
from concourse.bass_utils import run_bass_kernel_spmd

# problem constants (hardcoded per contest contract)
B, SEQ, DIM = 64, 256, 768
HEADS, KD, DV = 12, 64, 256
H = 4608
DH = 3072
EPS = 1e-5
SCALE = KD ** -0.5
NCORES = 8
BPC = B // NCORES          # batch elems per core
T = BPC * SEQ              # tokens per core = 2048
F32 = mybir.dt.float32
F32R = mybir.dt.float32r
BF16 = mybir.dt.bfloat16
ADD = mybir.AluOpType.add
MULT = mybir.AluOpType.mult
MIN = mybir.AluOpType.min


def _fused(tc, nc, xbf_t, wqk_t, wv_t, wp_t, bqk, bv, bsc, pg, pb, ones_c, y_t):
    with ExitStack() as ctx:
        res = ctx.enter_context(tc.tile_pool(name="res", bufs=1))
        xio = ctx.enter_context(tc.tile_pool(name="xio", bufs=2))
        qkio = ctx.enter_context(tc.tile_pool(name="qkio", bufs=2))
        vio = ctx.enter_context(tc.tile_pool(name="vio", bufs=2))
        work = ctx.enter_context(tc.tile_pool(name="work", bufs=3))
        hb = ctx.enter_context(tc.tile_pool(name="hb", bufs=2))
        yio = ctx.enter_context(tc.tile_pool(name="yio", bufs=2))
        ps = ctx.enter_context(tc.tile_pool(name="ps", bufs=1, space="PSUM"))

        # per-c-slice weight tiles: b0's first matmuls only wait on their own
        # slice's DMA, not the whole 23MB of resident weights
        wqk_l = [res.tile([128, 1536], BF16, name=f"wqk{c}") for c in range(6)]
        wv_l = [res.tile([128, DH], BF16, name=f"wv{c}") for c in range(6)]
        wp_sb = res.tile([128, 24, DIM], BF16)
        bsc_sb = res.tile([128, 12, 2, 256], BF16)   # additive attn bias
        bqk_sb = res.tile([128, 12], F32)
        bvb_sb = res.tile([128, DH], BF16)
        pg_sb = res.tile([128, 6], F32)
        pb_sb = res.tile([128, 6], F32)
        six_col = res.tile([128, 1], F32R)           # 6.0
        six_bf = res.tile([128, 1], BF16)
        one_row = res.tile([1, 128], F32R)           # 6.0 row (bc = 6*rcp = 1/sum)
        b3 = res.tile([128, 1], F32)

        def load_x(b):
            xb = xio.tile([128, 6, 256], BF16, name=f"xb{b}", tag="xb")
            bsl = slice(b * 256, (b + 1) * 256)
            nc.sync.dma_start(xb[:],
                              xbf_t.ap()[:, :, bsl].rearrange("c p n -> p c n"))
            return xb

        # first batch elem's x before the bulk weight traffic, so b0's first
        # matmuls only wait ~1us
        xb_first = load_x(0)
        nc.sync.dma_start(bqk_sb[:], bqk.ap())
        for c in range(6):
            nc.sync.dma_start(wqk_l[c][:], wqk_t.ap()[c])
            nc.sync.dma_start(wv_l[c][:], wv_t.ap()[c])
        nc.sync.dma_start(six_col[:], ones_c.ap())
        nc.vector.tensor_copy(six_bf[:], six_col[:])
        nc.sync.dma_start(one_row[:], ones_c.ap().rearrange("a b -> b a"))
        nc.vector.memset(b3[:], 3.0)
        nc.sync.dma_start(pg_sb[:], pg.ap())
        nc.sync.dma_start(pb_sb[:], pb.ap())
        for j in range(2):
            nc.sync.dma_start(bsc_sb[:, :, j, :],
                              bsc.ap()[:, j].rearrange("h p n -> p h n"))
        bv_ap = bv.ap()
        bv_bcast = bass.AP(tensor=bv_ap.tensor, offset=bv_ap.offset,
                           ap=[[0, 128]] + [list(p) for p in bv_ap.ap])
        nc.gpsimd.dma_start(bvb_sb[:], bv_bcast)
        for dq in range(4):
            nc.sync.dma_start(
                wp_sb[:, dq * 6:(dq + 1) * 6, :],
                wp_t.ap()[dq * 6:(dq + 1) * 6].rearrange("d p c -> p d c"))

        def emit_proj_group(hteff, bprev, ct, yst_prev):
            # proj GEMM group ct of batch elem bprev: y_T[c,:] = sum_d Wp_T.T h_T
            py = ps.tile([128, 256], F32, name=f"py{bprev}_{ct}", tag="py", bufs=1)
            for dt_ in range(24):
                nc.tensor.matmul(
                    py[:], wp_sb[:, dt_, ct * 128:(ct + 1) * 128], hteff[:, dt_, :],
                    start=(dt_ == 0), stop=(dt_ == 23))
            nc.vector.tensor_scalar(
                yst_prev[:, ct, :], py[:], pg_sb[:, ct:ct + 1], pb_sb[:, ct:ct + 1],
                MULT, ADD)
            if ct == 5:
                bslp = slice(bprev * 256, (bprev + 1) * 256)
                nc.sync.dma_start(
                    y_t.ap()[:, :, bslp].rearrange("c p n -> p c n"), yst_prev[:])

        sps_t = {}   # (b,h) -> psum tile with S scores
        es_t = {}    # (b,h) -> SBUF exp tile

        def emit_head_A(b, h, qk_sb):
            hb2, base = h // 2, 64 * (h % 2)
            psl = slice(base, base + 64)
            s_ps = ps.tile([128, 512], F32, name=f"sps{b}_{h}", tag="sps",
                           bufs=2)
            for jt in range(2):
                nc.scalar.copy(s_ps[:, jt * 256:(jt + 1) * 256],
                               bsc_sb[:, h, jt, :])
            for jt in range(2):
                nc.tensor.matmul(
                    s_ps[:, jt * 256:(jt + 1) * 256],
                    qk_sb[psl, 6 + hb2, jt * 128:(jt + 1) * 128],
                    qk_sb[psl, hb2, :],
                    start=False, stop=True)
            sps_t[(b, h)] = s_ps

        def emit_exp(b, h):
            es = work.tile([128, 512], BF16, name=f"es{b}_{h}", tag="es",
                           bufs=4)
            nc.scalar.activation(es[:], sps_t.pop((b, h))[:],
                                 mybir.ActivationFunctionType.Exp)
            es_t[(b, h)] = es

        def emit_head_B(b, h, v_b, h_t):
            # rowsum, 1/sum, O = v^T expS, hardswish. Emitted one section
            # after A so PE never waits on the exp.
            es = es_t.pop((b, h))
            rsbc = ps.tile([128, 512], F32, name=f"rsbc{b}_{h}", tag="rsbc",
                           bufs=1)
            for jt in range(2):
                nc.tensor.matmul(
                    rsbc[0:1, 0:256],
                    six_bf[:], es[:, jt * 256:(jt + 1) * 256],
                    start=(jt == 0), stop=(jt == 1))
            rcp = work.tile([1, 256], F32R, name=f"rcp{b}_{h}", tag="rcp",
                            bufs=2)
            with nc.allow_low_precision(reason="f32r rcp feeds bcast matmul"):
                nc.vector.reciprocal(rcp[:], rsbc[0:1, 0:256])   # = 1/(6*sum)
            # O matmuls before the bc matmul: PE stays busy while DVE computes
            # the reciprocal
            o_ps = ps.tile([128, 512], F32, name=f"o{b}_{h}", tag="o", bufs=2)
            for dvt in range(2):
                for jt in range(2):
                    nc.tensor.matmul(
                        o_ps[:, dvt * 256:(dvt + 1) * 256],
                        v_b[jt][:, h * 256 + dvt * 128:
                                h * 256 + (dvt + 1) * 128],
                        es[:, jt * 256:(jt + 1) * 256],
                        start=(jt == 0), stop=(jt == 1))
            # bc = 6*rcp = 1/sum via rank-1 matmul; stage to SBUF on ACT
            nc.tensor.matmul(rsbc[:, 256:512], one_row[:], rcp[:], start=True,
                             stop=True)
            bc_sb = work.tile([128, 256], F32, name=f"bcs{b}_{h}", tag="bcs",
                              bufs=2)
            nc.scalar.copy(bc_sb[:], rsbc[:, 256:512])
            for dvt in range(2):
                # y0 = O/sum; hswish(y0) = y0 * min(Relu(y0+3), 6) / 6
                y0 = work.tile([128, 256], BF16, name=f"y0{b}_{h}_{dvt}",
                               tag="y0", bufs=3)
                nc.vector.tensor_tensor(y0[:], o_ps[:, dvt * 256:(dvt + 1) * 256],
                                        bc_sb[:], MULT)
                r1 = work.tile([128, 256], BF16, name=f"r1{b}_{h}_{dvt}",
                               tag="r1", bufs=2)
                nc.scalar.activation(r1[:], y0[:],
                                     mybir.ActivationFunctionType.Relu,
                                     bias=b3[:], scale=1.0)
                m6 = work.tile([128, 256], BF16, name=f"m6{b}_{h}_{dvt}",
                               tag="m6", bufs=2)
                nc.vector.tensor_scalar(m6[:], r1[:], 6.0, 1.0 / 6.0, MIN, MULT)
                nc.vector.tensor_tensor(h_t[:, h * 2 + dvt, :], y0[:], m6[:],
                                        MULT)

        prev = None  # (h_t, b, yst) pending proj, pipelined one b behind
        xb_next = xb_first
        for b in range(BPC):
            xb = xb_next
            qk_sb = qkio.tile([128, 12, 256], BF16, name=f"qk{b}", tag="qk")
            v_b = [vio.tile([128, DH], BF16, name=f"vb{b}_{tt}", tag=f"vb{tt}",
                            bufs=2) for tt in range(2)]
            h_t = hb.tile([128, 24, 256], BF16, name=f"ht{b}", tag="ht")

            # head-pair sections, software-pipelined: section j runs qk/v GEMM
            # slice j, S matmuls for heads 2j/2j+1, exp for them, and the
            # B-stage (rowsum/O/hardswish) of the PREVIOUS pair, so every PE op
            # has its cross-engine inputs ready.
            for j in range(6):
                qps = ps.tile([128, 512], F32, name=f"qps{b}_{j}", tag="qps",
                              bufs=1)
                for half, cot in ((0, j), (1, 6 + j)):
                    for c in range(6):
                        nc.tensor.matmul(
                            qps[:, half * 256:(half + 1) * 256],
                            wqk_l[c][:, cot * 128:(cot + 1) * 128],
                            xb[:, c, :], start=(c == 0), stop=(c == 5))
                for half, cot in ((0, j), (1, 6 + j)):
                    nc.scalar.activation(
                        qk_sb[:, cot, :], qps[:, half * 256:(half + 1) * 256],
                        mybir.ActivationFunctionType.Identity,
                        bias=bqk_sb[:, cot:cot + 1], scale=1.0)
                for tt in range(2):
                    vps = ps.tile([128, 512], F32, name=f"vps{b}_{tt}_{j}",
                                  tag="vps", bufs=1)
                    for c in range(6):
                        nc.tensor.matmul(
                            vps[:], xb[:, c, tt * 128:(tt + 1) * 128],
                            wv_l[c][:, j * 512:(j + 1) * 512],
                            start=(c == 0), stop=(c == 5))
                    nc.vector.tensor_tensor(
                        v_b[tt][:, j * 512:(j + 1) * 512], vps[:],
                        bvb_sb[:, j * 512:(j + 1) * 512], ADD)
                emit_head_A(b, 2 * j, qk_sb)
                emit_head_A(b, 2 * j + 1, qk_sb)
                emit_exp(b, 2 * j)
                emit_exp(b, 2 * j + 1)
                if j == 2 and b + 1 < BPC:
                    xb_next = load_x(b + 1)
                if j > 0:
                    emit_head_B(b, 2 * (j - 1), v_b, h_t)
                    emit_head_B(b, 2 * (j - 1) + 1, v_b, h_t)
                    if prev is not None:
                        emit_proj_group(prev[0], prev[1], j - 1, prev[2])
            # drain the last pair + proj group of this b
            emit_head_B(b, 10, v_b, h_t)
            emit_head_B(b, 11, v_b, h_t)
            if prev is not None:
                emit_proj_group(prev[0], prev[1], 5, prev[2])

            yst = yio.tile([128, 6, 256], F32, name=f"yst{b}", tag="yst")
            prev = (h_t, b, yst)

        # drain the last b's proj
        for ct in range(6):
            emit_proj_group(prev[0], prev[1], ct, prev[2])


def _build(reps=1, phase="both"):
    nc = bacc.Bacc("TRN2", target_bir_lowering=False, debug=False)
    xbf_t = nc.dram_tensor("xbf_t", [6, 128, T], BF16, kind="ExternalInput")
    wqk_t = nc.dram_tensor("wqk_t", [6, 128, 1536], BF16, kind="ExternalInput")
    wv_t = nc.dram_tensor("wv_t", [6, 128, DH], BF16, kind="ExternalInput")
    wp_t = nc.dram_tensor("wp_t", [24, 128, DIM], BF16, kind="ExternalInput")
    bqk = nc.dram_tensor("bqk", [128, 12], F32, kind="ExternalInput")
    bv = nc.dram_tensor("bv", [DH], BF16, kind="ExternalInput")
    bsc = nc.dram_tensor("bsc", [12, 2, 128, 256], BF16, kind="ExternalInput")
    pg = nc.dram_tensor("pg", [128, 6], F32, kind="ExternalInput")
    pb = nc.dram_tensor("pb", [128, 6], F32, kind="ExternalInput")
    y_t = nc.dram_tensor("y_t", [6, 128, T], F32, kind="ExternalOutput")
    ones_c = nc.dram_tensor("ones_c", [128, 1], F32R, kind="ExternalInput")

    with tile.TileContext(nc) as tc:
        if reps == 1:
            _fused(tc, nc, xbf_t, wqk_t, wv_t, wp_t, bqk, bv, bsc, pg, pb,
                   ones_c, y_t)
        else:
            with tc.For_i(0, reps, 1):
                _fused(tc, nc, xbf_t, wqk_t, wv_t, wp_t, bqk, bv, bsc, pg, pb,
                       ones_c, y_t)
    nc.compile()
    return nc


_NC = None


def _get_nc():
    global _NC
    if _NC is None:
        _NC = _build()
    return _NC


def _prep_host(qkv_w, qkv_gamma, qkv_beta, qkv_mean, qkv_var,
               attn_biases, proj_w, proj_gamma, proj_beta, proj_mean, proj_var,
               bias_idxs):
    f32 = np.float32
    bf16 = ml_dtypes.bfloat16
    qkv_w = np.asarray(qkv_w, f32)
    s = np.asarray(qkv_gamma, f32) / np.sqrt(np.asarray(qkv_var, f32) + EPS)
    Wf = qkv_w * s[:, None]
    bf = np.asarray(qkv_beta, f32) - np.asarray(qkv_mean, f32) * s

    # channel order: cot 0-5 = q blocks (heads 2j,2j+1 per 128), 6-11 = k blocks
    base = np.arange(HEADS, dtype=np.int64) * 384
    q_ch = (base[:, None] + np.arange(64)[None, :]).reshape(-1)          # q rows
    k_ch = (base[:, None] + 64 + np.arange(64)[None, :]).reshape(-1)     # k rows
    v_ch = (base[:, None] + 128 + np.arange(256)[None, :]).reshape(-1)
    qk_ch = np.concatenate([q_ch, k_ch])

    Wqk = Wf[qk_ch].copy()
    bqk_v = bf[qk_ch].copy()
    Wqk[:768] *= SCALE      # fold softmax scale into q
    bqk_v[:768] *= SCALE

    wqk_t = np.ascontiguousarray(Wqk.T).reshape(6, 128, 1536).astype(bf16)
    wv_t = np.ascontiguousarray(Wf[v_ch].T).reshape(6, 128, DH).astype(bf16)
    bqk_np = np.ascontiguousarray(bqk_v.reshape(12, 128).T)
    bv_np = bf[v_ch].astype(bf16)

    proj_w = np.asarray(proj_w, f32)
    sp = np.asarray(proj_gamma, f32) / np.sqrt(np.asarray(proj_var, f32) + EPS)
    bp_v = np.asarray(proj_beta, f32) - np.asarray(proj_mean, f32) * sp
    wp_t = np.ascontiguousarray(proj_w.T).reshape(24, 128, DIM).astype(bf16)
    pg_np = np.ascontiguousarray(sp.reshape(6, 128).T)
    pb_np = np.ascontiguousarray(bp_v.reshape(6, 128).T)

    bias_full = np.asarray(attn_biases, f32)[:, np.asarray(bias_idxs)]  # [h,i,j]
    bsc_np = np.ascontiguousarray(
        bias_full.transpose(0, 2, 1)).reshape(HEADS, 2, 128, 256).astype(bf16)

    return dict(wqk_t=wqk_t, wv_t=wv_t, bqk=bqk_np, bv=bv_np,
                wp_t=wp_t, pg=pg_np, pb=pb_np, bsc=bsc_np,
                ones_c=np.full((128, 1), 6.0, f32))


def kernel(x, qkv_w, qkv_gamma, qkv_beta, qkv_mean, qkv_var,
           attn_biases, proj_w, proj_gamma, proj_beta, proj_mean, proj_var,
           bias_idxs):
    x = np.asarray(x, np.float32)
    shared = _prep_host(qkv_w, qkv_gamma, qkv_beta, qkv_mean, qkv_var,
                        attn_biases, proj_w, proj_gamma, proj_beta,
                        proj_mean, proj_var, bias_idxs)
    in_maps = []
    for ci in range(NCORES):
        xc = x[ci * BPC:(ci + 1) * BPC].reshape(T, DIM)
        x_tc = np.ascontiguousarray(xc.T).reshape(6, 128, T).astype(
            ml_dtypes.bfloat16)
        m = dict(shared)
        m["xbf_t"] = x_tc
        in_maps.append(m)

    nc = _get_nc()
    res = run_bass_kernel_spmd(nc, in_maps, core_ids=list(range(NCORES)))

    out = np.empty((B, SEQ, DIM), np.float32)
    for ci in range(NCORES):
        yt = np.asarray(res.results[ci]["y_t"]).reshape(DIM, T)
        out[ci * BPC:(ci + 1) * BPC] = yt.T.reshape(BPC, SEQ, DIM)
    return out


# revision 36
# speedup vs baseline: 1.1626x; 1.1626x over previous
"""Trainium2 Bass kernel for nn_Attention_46901042872659.

Dense transformer attention block:
  qkv = BN(x @ qkv_w.T); split q,k,v per head; attn = softmax(q k^T * scale + bias);
  out = hardswish(attn @ v); y = BN(out @ proj_w.T)

Strategy: data-parallel over batch across 8 NeuronCores (8 batch elems each).
Fully fused per batch element: qkv GEMM -> attention -> proj all stay on-chip
(no DRAM scratch). qkv/proj GEMMs and attention matmuls run bf16 (full PE
rate); PSUM accumulation is f32. Eval-mode BN is folded into GEMM weights/bias
on the host; softmax scale folded into q weights; relative-position bias
gathered host-side into a dense [heads, j, i] matrix.

Per-core program (SPMD on 8 cores), per batch elem b (8 per core):
  - qk GEMM: out channel-major [co 128, t 256]; cot 0-5 = q blocks (heads
    2j/2j+1 in partition halves), cot 6-11 = k blocks. Bias rides ACT.
  - v GEMM: out token-major [t 128, co 3072]; bias rides the DVE mover.
  - per head: S psum preloaded with attn bias (ACT copy), S += k^T q;
    exp on ACT; rowsum via ones-matmul; 1/rowsum (DVE); partition-broadcast
    via rank-1 PE matmul staged to SBUF on ACT; O = v^T expS;
    hardswish via relu6 form: h = y0 * min(Relu(y0+3), 6)/6, y0 = O/rowsum.
  - proj GEMM interleaved one batch-elem behind; BN fold on DVE.
"""
import numpy as np
import ml_dtypes
from contextlib import ExitStack

import concourse.bass as bass
import concourse.tile as tile
from concourse import bacc, bass_isa, mybir
from concourse.bass_utils import run_bass_kernel_spmd

# problem constants (hardcoded per contest contract)
B, SEQ, DIM = 64, 256, 768
HEADS, KD, DV = 12, 64, 256
H = 4608
DH = 3072
EPS = 1e-5
SCALE = KD ** -0.5
NCORES = 8
BPC = B // NCORES          # batch elems per core
T = BPC * SEQ              # tokens per core = 2048
F32 = mybir.dt.float32
F32R = mybir.dt.float32r
BF16 = mybir.dt.bfloat16
ADD = mybir.AluOpType.add
MULT = mybir.AluOpType.mult
MIN = mybir.AluOpType.min


def _fused(tc, nc, xbf_t, wqk_t, wv_t, wp_t, bqk, bv, bsc, pg, pb, ones_c, y_t):
    with ExitStack() as ctx:
        res = ctx.enter_context(tc.tile_pool(name="res", bufs=1))
        xio = ctx.enter_context(tc.tile_pool(name="xio", bufs=2))
        qkio = ctx.enter_context(tc.tile_pool(name="qkio", bufs=2))
        vio = ctx.enter_context(tc.tile_pool(name="vio", bufs=2))
        work = ctx.enter_context(tc.tile_pool(name="work", bufs=3))
        hb = ctx.enter_context(tc.tile_pool(name="hb", bufs=2))
        yio = ctx.enter_context(tc.tile_pool(name="yio", bufs=2))
        ps = ctx.enter_context(tc.tile_pool(name="ps", bufs=1, space="PSUM"))

        # per-c-slice weight tiles: b0's first matmuls only wait on their own
        # slice's DMA, not the whole 23MB of resident weights
        wqk_l = [res.tile([128, 1536], BF16, name=f"wqk{c}") for c in range(6)]
        wv_l = [res.tile([128, DH], BF16, name=f"wv{c}") for c in range(6)]
        wp_sb = res.tile([128, 24, DIM], BF16)
        bsc_sb = res.tile([128, 12, 2, 256], BF16)   # additive attn bias
        bqk_sb = res.tile([128, 12], F32)
        bvb_sb = res.tile([128, DH], BF16)
        pg_sb = res.tile([128, 6], F32)
        pb_sb = res.tile([128, 6], F32)
        six_col = res.tile([128, 1], F32R)           # 6.0
        six_bf = res.tile([128, 1], BF16)
        one_row = res.tile([1, 128], F32R)           # 6.0 row (bc = 6*rcp = 1/sum)
        b3 = res.tile([128, 1], F32)

        def load_x(b):
            xb = xio.tile([128, 6, 256], BF16, name=f"xb{b}", tag="xb")
            bsl = slice(b * 256, (b + 1) * 256)
            nc.sync.dma_start(xb[:],
                              xbf_t.ap()[:, :, bsl].rearrange("c p n -> p c n"))
            return xb

        # first batch elem's x before the bulk weight traffic, so b0's first
        # matmuls only wait ~1us
        xb_first = load_x(0)
        nc.sync.dma_start(bqk_sb[:], bqk.ap())
        for c in range(6):
            nc.sync.dma_start(wqk_l[c][:], wqk_t.ap()[c])
            nc.sync.dma_start(wv_l[c][:], wv_t.ap()[c])
        nc.sync.dma_start(six_col[:], ones_c.ap())
        nc.vector.tensor_copy(six_bf[:], six_col[:])
        nc.sync.dma_start(one_row[:], ones_c.ap().rearrange("a b -> b a"))
        nc.vector.memset(b3[:], 3.0)
        nc.sync.dma_start(pg_sb[:], pg.ap())
        nc.sync.dma_start(pb_sb[:], pb.ap())
        for j in range(2):
            nc.sync.dma_start(bsc_sb[:, :, j, :],
                              bsc.ap()[:, j].rearrange("h p n -> p h n"))
        bv_ap = bv.ap()
        bv_bcast = bass.AP(tensor=bv_ap.tensor, offset=bv_ap.offset,
                           ap=[[0, 128]] + [list(p) for p in bv_ap.ap])
        nc.gpsimd.dma_start(bvb_sb[:], bv_bcast)
        for dq in range(4):
            nc.sync.dma_start(
                wp_sb[:, dq * 6:(dq + 1) * 6, :],
                wp_t.ap()[dq * 6:(dq + 1) * 6].rearrange("d p c -> p d c"))

        def emit_proj_group(hteff, bprev, ct, yst_prev):
            # proj GEMM group ct of batch elem bprev: y_T[c,:] = sum_d Wp_T.T h_T
            py = ps.tile([128, 256], F32, name=f"py{bprev}_{ct}", tag="py", bufs=1)
            for dt_ in range(24):
                nc.tensor.matmul(
                    py[:], wp_sb[:, dt_, ct * 128:(ct + 1) * 128], hteff[:, dt_, :],
                    start=(dt_ == 0), stop=(dt_ == 23))
            nc.vector.tensor_scalar(
                yst_prev[:, ct, :], py[:], pg_sb[:, ct:ct + 1], pb_sb[:, ct:ct + 1],
                MULT, ADD)
            if ct == 5:
                bslp = slice(bprev * 256, (bprev + 1) * 256)
                nc.sync.dma_start(
                    y_t.ap()[:, :, bslp].rearrange("c p n -> p c n"), yst_prev[:])

        sps_t = {}   # (b,h) -> psum tile with S scores
        es_t = {}    # (b,h) -> SBUF exp tile

        def emit_head_A(b, h, qk_sb):
            hb2, base = h // 2, 64 * (h % 2)
            psl = slice(base, base + 64)
            s_ps = ps.tile([128, 512], F32, name=f"sps{b}_{h}", tag="sps",
                           bufs=2)
            for jt in range(2):
                nc.scalar.copy(s_ps[:, jt * 256:(jt + 1) * 256],
                               bsc_sb[:, h, jt, :])
            for jt in range(2):
                nc.tensor.matmul(
                    s_ps[:, jt * 256:(jt + 1) * 256],
                    qk_sb[psl, 6 + hb2, jt * 128:(jt + 1) * 128],
                    qk_sb[psl, hb2, :],
                    start=False, stop=True)
            sps_t[(b, h)] = s_ps

        def emit_exp(b, h):
            es = work.tile([128, 512], BF16, name=f"es{b}_{h}", tag="es",
                           bufs=4)
            nc.scalar.activation(es[:], sps_t.pop((b, h))[:],
                                 mybir.ActivationFunctionType.Exp)
            es_t[(b, h)] = es

        def emit_head_B(b, h, v_b, h_t):
            # rowsum, 1/sum, O = v^T expS, hardswish. Emitted one section
            # after A so PE never waits on the exp.
            es = es_t.pop((b, h))
            rs_ps = ps.tile([1, 256], F32, name=f"rs{b}_{h}", tag="rs",
                            bufs=1)
            for jt in range(2):
                nc.tensor.matmul(
                    rs_ps[:],
                    six_bf[:], es[:, jt * 256:(jt + 1) * 256],
                    start=(jt == 0), stop=(jt == 1))
            rcp = work.tile([1, 256], F32, name=f"rcp{b}_{h}", tag="rcp",
                            bufs=2)
            nc.vector.reciprocal(rcp[:], rs_ps[:])   # = 1/(6*sum)
            o_ps = ps.tile([128, 512], F32, name=f"o{b}_{h}", tag="o", bufs=2)
            for dvt in range(2):
                for jt in range(2):
                    nc.tensor.matmul(
                        o_ps[:, dvt * 256:(dvt + 1) * 256],
                        v_b[jt][:, h * 256 + dvt * 128:
                                h * 256 + (dvt + 1) * 128],
                        es[:, jt * 256:(jt + 1) * 256],
                        start=(jt == 0), stop=(jt == 1))
            # partition-broadcast of rcp on GPSIMD: the only gpsimd compute op
            # in the program, so its library loads exactly once (no thrash).
            # bc = 6 * rcp = 1/sum after the DVE scale below
            bc_sb = work.tile([128, 256], F32, name=f"bcs{b}_{h}", tag="bcs",
                              bufs=2)
            nc.gpsimd.partition_broadcast(bc_sb[:], rcp[:])
            for dvt in range(2):
                # y0 = O/(6*sum); hswish = y0 * min(Relu(6*y0+3), 6)
                y0 = work.tile([128, 256], BF16, name=f"y0{b}_{h}_{dvt}",
                               tag="y0", bufs=3)
                nc.vector.tensor_tensor(y0[:], o_ps[:, dvt * 256:(dvt + 1) * 256],
                                        bc_sb[:], MULT)
                r1 = work.tile([128, 256], BF16, name=f"r1{b}_{h}_{dvt}",
                               tag="r1", bufs=2)
                nc.scalar.activation(r1[:], y0[:],
                                     mybir.ActivationFunctionType.Relu,
                                     bias=b3[:], scale=6.0)
                m6 = work.tile([128, 256], BF16, name=f"m6{b}_{h}_{dvt}",
                               tag="m6", bufs=2)
                nc.vector.tensor_scalar_min(m6[:], r1[:], 6.0)
                nc.vector.tensor_tensor(h_t[:, h * 2 + dvt, :], y0[:], m6[:],
                                        MULT)

        prev = None  # (h_t, b, yst) pending proj, pipelined one b behind
        xb_next = xb_first
        for b in range(BPC):
            xb = xb_next
            qk_sb = qkio.tile([128, 12, 256], BF16, name=f"qk{b}", tag="qk")
            v_b = [vio.tile([128, DH], BF16, name=f"vb{b}_{tt}", tag=f"vb{tt}",
                            bufs=2) for tt in range(2)]
            h_t = hb.tile([128, 24, 256], BF16, name=f"ht{b}", tag="ht")

            # head-pair sections, software-pipelined: section j runs qk/v GEMM
            # slice j, S matmuls for heads 2j/2j+1, exp for them, and the
            # B-stage (rowsum/O/hardswish) of the PREVIOUS pair, so every PE op
            # has its cross-engine inputs ready.
            for j in range(6):
                qps = ps.tile([128, 512], F32, name=f"qps{b}_{j}", tag="qps",
                              bufs=1)
                for half, cot in ((0, j), (1, 6 + j)):
                    for c in range(6):
                        nc.tensor.matmul(
                            qps[:, half * 256:(half + 1) * 256],
                            wqk_l[c][:, cot * 128:(cot + 1) * 128],
                            xb[:, c, :], start=(c == 0), stop=(c == 5))
                for half, cot in ((0, j), (1, 6 + j)):
                    nc.scalar.activation(
                        qk_sb[:, cot, :], qps[:, half * 256:(half + 1) * 256],
                        mybir.ActivationFunctionType.Identity,
                        bias=bqk_sb[:, cot:cot + 1], scale=1.0)
                for tt in range(2):
                    vps = ps.tile([128, 512], F32, name=f"vps{b}_{tt}_{j}",
                                  tag="vps", bufs=1)
                    for c in range(6):
                        nc.tensor.matmul(
                            vps[:], xb[:, c, tt * 128:(tt + 1) * 128],
                            wv_l[c][:, j * 512:(j + 1) * 512],
                            start=(c == 0), stop=(c == 5))
                    nc.vector.tensor_tensor(
                        v_b[tt][:, j * 512:(j + 1) * 512], vps[:],
                        bvb_sb[:, j * 512:(j + 1) * 512], ADD)
                emit_head_A(b, 2 * j, qk_sb)
                emit_head_A(b, 2 * j + 1, qk_sb)
                emit_exp(b, 2 * j)
                emit_exp(b, 2 * j + 1)
                if j == 2 and b + 1 < BPC:
                    xb_next = load_x(b + 1)
                if j > 0:
                    emit_head_B(b, 2 * (j - 1), v_b, h_t)
                    emit_head_B(b, 2 * (j - 1) + 1, v_b, h_t)
                    if prev is not None:
                        emit_proj_group(prev[0], prev[1], j - 1, prev[2])
            # drain the last pair + proj group of this b
            emit_head_B(b, 10, v_b, h_t)
            emit_head_B(b, 11, v_b, h_t)
            if prev is not None:
                emit_proj_group(prev[0], prev[1], 5, prev[2])

            yst = yio.tile([128, 6, 256], F32, name=f"yst{b}", tag="yst")
            prev = (h_t, b, yst)

        # drain the last b's proj
        for ct in range(6):
            emit_proj_group(prev[0], prev[1], ct, prev[2])


def _build(reps=1, phase="both"):
    nc = bacc.Bacc("TRN2", target_bir_lowering=False, debug=False)
    xbf_t = nc.dram_tensor("xbf_t", [6, 128, T], BF16, kind="ExternalInput")
    wqk_t = nc.dram_tensor("wqk_t", [6, 128, 1536], BF16, kind="ExternalInput")
    wv_t = nc.dram_tensor("wv_t", [6, 128, DH], BF16, kind="ExternalInput")
    wp_t = nc.dram_tensor("wp_t", [24, 128, DIM], BF16, kind="ExternalInput")
    bqk = nc.dram_tensor("bqk", [128, 12], F32, kind="ExternalInput")
    bv = nc.dram_tensor("bv", [DH], BF16, kind="ExternalInput")
    bsc = nc.dram_tensor("bsc", [12, 2, 128, 256], BF16, kind="ExternalInput")
    pg = nc.dram_tensor("pg", [128, 6], F32, kind="ExternalInput")
    pb = nc.dram_tensor("pb", [128, 6], F32, kind="ExternalInput")
    y_t = nc.dram_tensor("y_t", [6, 128, T], F32, kind="ExternalOutput")
    ones_c = nc.dram_tensor("ones_c", [128, 1], F32R, kind="ExternalInput")

    with tile.TileContext(nc) as tc:
        if reps == 1:
            _fused(tc, nc, xbf_t, wqk_t, wv_t, wp_t, bqk, bv, bsc, pg, pb,
                   ones_c, y_t)
        else:
            with tc.For_i(0, reps, 1):
                _fused(tc, nc, xbf_t, wqk_t, wv_t, wp_t, bqk, bv, bsc, pg, pb,
                       ones_c, y_t)
    nc.compile()
    return nc


_NC = None


def _get_nc():
    global _NC
    if _NC is None:
        _NC = _build()
    return _NC


def _prep_host(qkv_w, qkv_gamma, qkv_beta, qkv_mean, qkv_var,
               attn_biases, proj_w, proj_gamma, proj_beta, proj_mean, proj_var,
               bias_idxs):
    f32 = np.float32
    bf16 = ml_dtypes.bfloat16
    qkv_w = np.asarray(qkv_w, f32)
    s = np.asarray(qkv_gamma, f32) / np.sqrt(np.asarray(qkv_var, f32) + EPS)
    Wf = qkv_w * s[:, None]
    bf = np.asarray(qkv_beta, f32) - np.asarray(qkv_mean, f32) * s

    # channel order: cot 0-5 = q blocks (heads 2j,2j+1 per 128), 6-11 = k blocks
    base = np.arange(HEADS, dtype=np.int64) * 384
    q_ch = (base[:, None] + np.arange(64)[None, :]).reshape(-1)          # q rows
    k_ch = (base[:, None] + 64 + np.arange(64)[None, :]).reshape(-1)     # k rows
    v_ch = (base[:, None] + 128 + np.arange(256)[None, :]).reshape(-1)
    qk_ch = np.concatenate([q_ch, k_ch])

    Wqk = Wf[qk_ch].copy()
    bqk_v = bf[qk_ch].copy()
    Wqk[:768] *= SCALE      # fold softmax scale into q
    bqk_v[:768] *= SCALE

    wqk_t = np.ascontiguousarray(Wqk.T).reshape(6, 128, 1536).astype(bf16)
    wv_t = np.ascontiguousarray(Wf[v_ch].T).reshape(6, 128, DH).astype(bf16)
    bqk_np = np.ascontiguousarray(bqk_v.reshape(12, 128).T)
    bv_np = bf[v_ch].astype(bf16)

    proj_w = np.asarray(proj_w, f32)
    sp = np.asarray(proj_gamma, f32) / np.sqrt(np.asarray(proj_var, f32) + EPS)
    bp_v = np.asarray(proj_beta, f32) - np.asarray(proj_mean, f32) * sp
    wp_t = np.ascontiguousarray(proj_w.T).reshape(24, 128, DIM).astype(bf16)
    pg_np = np.ascontiguousarray(sp.reshape(6, 128).T)
    pb_np = np.ascontiguousarray(bp_v.reshape(6, 128).T)

    bias_full = np.asarray(attn_biases, f32)[:, np.asarray(bias_idxs)]  # [h,i,j]
    bsc_np = np.ascontiguousarray(
        bias_full.transpose(0, 2, 1)).reshape(HEADS, 2, 128, 256).astype(bf16)

    return dict(wqk_t=wqk_t, wv_t=wv_t, bqk=bqk_np, bv=bv_np,
                wp_t=wp_t, pg=pg_np, pb=pb_np, bsc=bsc_np,
                ones_c=np.full((128, 1), 6.0, f32))


def kernel(x, qkv_w, qkv_gamma, qkv_beta, qkv_mean, qkv_var,
           attn_biases, proj_w, proj_gamma, proj_beta, proj_mean, proj_var,
           bias_idxs):
    x = np.asarray(x, np.float32)
    shared = _prep_host(qkv_w, qkv_gamma, qkv_beta, qkv_mean, qkv_var,
                        attn_biases, proj_w, proj_gamma, proj_beta,
                        proj_mean, proj_var, bias_idxs)
    in_maps = []
    for ci in range(NCORES):
        xc = x[ci * BPC:(ci + 1) * BPC].reshape(T, DIM)
        x_tc = np.ascontiguousarray(xc.T).reshape(6, 128, T).astype(
            ml_dtypes.bfloat16)
        m = dict(shared)
        m["xbf_t"] = x_tc
        in_maps.append(m)

    nc = _get_nc()
    res = run_bass_kernel_spmd(nc, in_maps, core_ids=list(range(NCORES)))

    out = np.empty((B, SEQ, DIM), np.float32)
    for ci in range(NCORES):
        yt = np.asarray(res.results[ci]["y_t"]).reshape(DIM, T)
        out[ci * BPC:(ci + 1) * BPC] = yt.T.reshape(BPC, SEQ, DIM)
    return out


# revision 41
# speedup vs baseline: 1.2036x; 1.0353x over previous
"""Trainium2 Bass kernel for nn_Attention_46901042872659.

Dense transformer attention block:
  qkv = BN(x @ qkv_w.T); split q,k,v per head; attn = softmax(q k^T * scale + bias);
  out = hardswish(attn @ v); y = BN(out @ proj_w.T)

Strategy: data-parallel over batch across 8 NeuronCores (8 batch elems each).
Fully fused per batch element: qkv GEMM -> attention -> proj all stay on-chip
(no DRAM scratch). qkv/proj GEMMs and attention matmuls run bf16 (full PE
rate); PSUM accumulation is f32. Eval-mode BN is folded into GEMM weights/bias
on the host; softmax scale folded into q weights; relative-position bias
gathered host-side into a dense [heads, j, i] matrix.

Per-core program (SPMD on 8 cores), per batch elem b (8 per core):
  - qk GEMM: out channel-major [co 128, t 256]; cot 0-5 = q blocks (heads
    2j/2j+1 in partition halves), cot 6-11 = k blocks. Bias rides ACT.
  - v GEMM: out token-major [t 128, co 3072]; bias rides the DVE mover.
  - per head (software-pipelined one section deep): S psum preloaded with
    attn bias (ACT copy), S += k^T q; exp on ACT; rowsum via ones-matmul;
    1/rowsum (DVE); partition-broadcast on GPSIMD (sole gpsimd compute op,
    so its library loads once); O = v^T expS; hardswish via relu6 form:
    h = y0 * min(Relu(6*y0+3), 6), y0 = O/(6*rowsum).
  - proj GEMM interleaved one batch-elem behind; BN fold on DVE.
"""
import numpy as np
import ml_dtypes
from contextlib import ExitStack

import concourse.bass as bass
import concourse.tile as tile
from concourse import bacc, bass_isa, mybir
from concourse.bass_utils import run_bass_kernel_spmd

# problem constants (hardcoded per contest contract)
B, SEQ, DIM = 64, 256, 768
HEADS, KD, DV = 12, 64, 256
H = 4608
DH = 3072
EPS = 1e-5
SCALE = KD ** -0.5
NCORES = 8
BPC = B // NCORES          # batch elems per core
T = BPC * SEQ              # tokens per core = 2048
F32 = mybir.dt.float32
F32R = mybir.dt.float32r
BF16 = mybir.dt.bfloat16
ADD = mybir.AluOpType.add
MULT = mybir.AluOpType.mult
MIN = mybir.AluOpType.min


def _fused(tc, nc, xbf_t, wqk_t, wv_t, wp_t, bqk, bv, bsc, pg, pb, ones_c, y_t):
    with ExitStack() as ctx:
        res = ctx.enter_context(tc.tile_pool(name="res", bufs=1))
        xio = ctx.enter_context(tc.tile_pool(name="xio", bufs=2))
        qkio = ctx.enter_context(tc.tile_pool(name="qkio", bufs=2))
        vio = ctx.enter_context(tc.tile_pool(name="vio", bufs=2))
        work = ctx.enter_context(tc.tile_pool(name="work", bufs=3))
        hb = ctx.enter_context(tc.tile_pool(name="hb", bufs=2))
        yio = ctx.enter_context(tc.tile_pool(name="yio", bufs=2))
        ps = ctx.enter_context(tc.tile_pool(name="ps", bufs=1, space="PSUM"))

        # per-c-slice weight tiles: b0's first matmuls only wait on their own
        # slice's DMA, not the whole 23MB of resident weights
        wqk_l = [res.tile([128, 1536], BF16, name=f"wqk{c}") for c in range(6)]
        wv_l = [res.tile([128, DH], BF16, name=f"wv{c}") for c in range(6)]
        wp_sb = res.tile([128, 24, DIM], BF16)
        bsc_sb = res.tile([128, 12, 2, 256], BF16)   # additive attn bias
        bqk_sb = res.tile([128, 12], F32)
        bvb_sb = res.tile([128, DH], BF16)
        pg_sb = res.tile([128, 6], F32)
        pb_sb = res.tile([128, 6], F32)
        six_col = res.tile([128, 1], F32R)           # 6.0
        six_bf = res.tile([128, 1], BF16)
        one_row = res.tile([1, 128], F32R)           # 6.0 row (bc = 6*rcp = 1/sum)
        b3 = res.tile([128, 1], F32)

        def load_x(b):
            xb = xio.tile([128, 6, 256], BF16, name=f"xb{b}", tag="xb")
            bsl = slice(b * 256, (b + 1) * 256)
            nc.sync.dma_start(xb[:],
                              xbf_t.ap()[:, :, bsl].rearrange("c p n -> p c n"))
            return xb

        # first batch elem's x before the bulk weight traffic, so b0's first
        # matmuls only wait ~1us
        xb_first = load_x(0)
        nc.sync.dma_start(bqk_sb[:], bqk.ap())
        for c in range(6):
            nc.sync.dma_start(wqk_l[c][:], wqk_t.ap()[c])
            nc.sync.dma_start(wv_l[c][:], wv_t.ap()[c])
        nc.sync.dma_start(six_col[:], ones_c.ap())
        nc.vector.tensor_copy(six_bf[:], six_col[:])
        nc.sync.dma_start(one_row[:], ones_c.ap().rearrange("a b -> b a"))
        nc.vector.memset(b3[:], 3.0)
        nc.sync.dma_start(pg_sb[:], pg.ap())
        nc.sync.dma_start(pb_sb[:], pb.ap())
        for j in range(2):
            nc.sync.dma_start(bsc_sb[:, :, j, :],
                              bsc.ap()[:, j].rearrange("h p n -> p h n"))
        bv_ap = bv.ap()
        bv_bcast = bass.AP(tensor=bv_ap.tensor, offset=bv_ap.offset,
                           ap=[[0, 128]] + [list(p) for p in bv_ap.ap])
        nc.gpsimd.dma_start(bvb_sb[:], bv_bcast)
        for dq in range(4):
            nc.sync.dma_start(
                wp_sb[:, dq * 6:(dq + 1) * 6, :],
                wp_t.ap()[dq * 6:(dq + 1) * 6].rearrange("d p c -> p d c"))

        def emit_proj_group(hteff, bprev, ct, yst_prev):
            # proj GEMM group ct of batch elem bprev: y_T[c,:] = sum_d Wp_T.T h_T
            py = ps.tile([128, 256], F32, name=f"py{bprev}_{ct}", tag="py", bufs=1)
            for dt_ in range(24):
                nc.tensor.matmul(
                    py[:], wp_sb[:, dt_, ct * 128:(ct + 1) * 128], hteff[:, dt_, :],
                    start=(dt_ == 0), stop=(dt_ == 23))
            nc.vector.tensor_scalar(
                yst_prev[:, ct, :], py[:], pg_sb[:, ct:ct + 1], pb_sb[:, ct:ct + 1],
                MULT, ADD)
            if ct == 5:
                bslp = slice(bprev * 256, (bprev + 1) * 256)
                nc.sync.dma_start(
                    y_t.ap()[:, :, bslp].rearrange("c p n -> p c n"), yst_prev[:])

        sps_t = {}   # (b,h) -> psum tile with S scores
        es_t = {}    # (b,h) -> SBUF exp tile

        def emit_head_A(b, h, qk_sb):
            hb2, base = h // 2, 64 * (h % 2)
            psl = slice(base, base + 64)
            s_ps = ps.tile([128, 512], F32, name=f"sps{b}_{h}", tag="sps",
                           bufs=2)
            for jt in range(2):
                nc.scalar.copy(s_ps[:, jt * 256:(jt + 1) * 256],
                               bsc_sb[:, h, jt, :])
            for jt in range(2):
                nc.tensor.matmul(
                    s_ps[:, jt * 256:(jt + 1) * 256],
                    qk_sb[psl, 6 + hb2, jt * 128:(jt + 1) * 128],
                    qk_sb[psl, hb2, :],
                    start=False, stop=True)
            sps_t[(b, h)] = s_ps

        def emit_exp(b, h):
            es = work.tile([128, 512], BF16, name=f"es{b}_{h}", tag="es",
                           bufs=4)
            nc.scalar.activation(es[:], sps_t.pop((b, h))[:],
                                 mybir.ActivationFunctionType.Exp)
            es_t[(b, h)] = es

        def emit_head_B(b, h, v_b, h_t):
            # rowsum, 1/sum, O = v^T expS, hardswish. Emitted one section
            # after A so PE never waits on the exp.
            es = es_t.pop((b, h))
            rs_ps = ps.tile([1, 256], F32, name=f"rs{b}_{h}", tag="rs",
                            bufs=1)
            for jt in range(2):
                nc.tensor.matmul(
                    rs_ps[:],
                    six_bf[:], es[:, jt * 256:(jt + 1) * 256],
                    start=(jt == 0), stop=(jt == 1))
            rcp = work.tile([1, 256], F32, name=f"rcp{b}_{h}", tag="rcp",
                            bufs=2)
            nc.vector.reciprocal(rcp[:], rs_ps[:])   # = 1/(6*sum)
            o_ps = ps.tile([128, 512], F32, name=f"o{b}_{h}", tag="o", bufs=2)
            for dvt in range(2):
                for jt in range(2):
                    nc.tensor.matmul(
                        o_ps[:, dvt * 256:(dvt + 1) * 256],
                        v_b[jt][:, h * 256 + dvt * 128:
                                h * 256 + (dvt + 1) * 128],
                        es[:, jt * 256:(jt + 1) * 256],
                        start=(jt == 0), stop=(jt == 1))
            # partition-broadcast of rcp on GPSIMD: the only gpsimd compute op
            # in the program, so its library loads exactly once (no thrash).
            # bc = 6 * rcp = 1/sum after the DVE scale below
            bc_sb = work.tile([128, 256], F32, name=f"bcs{b}_{h}", tag="bcs",
                              bufs=2)
            nc.gpsimd.partition_broadcast(bc_sb[:], rcp[:])
            for dvt in range(2):
                # y0 = O/(6*sum); hswish = y0 * min(Relu(6*y0+3), 6)
                y0 = work.tile([128, 256], BF16, name=f"y0{b}_{h}_{dvt}",
                               tag="y0", bufs=3)
                nc.vector.tensor_tensor(y0[:], o_ps[:, dvt * 256:(dvt + 1) * 256],
                                        bc_sb[:], MULT)
                r1 = work.tile([128, 256], BF16, name=f"r1{b}_{h}_{dvt}",
                               tag="r1", bufs=2)
                nc.scalar.activation(r1[:], y0[:],
                                     mybir.ActivationFunctionType.Relu,
                                     bias=b3[:], scale=6.0)
                m6 = work.tile([128, 256], BF16, name=f"m6{b}_{h}_{dvt}",
                               tag="m6", bufs=2)
                nc.vector.tensor_scalar_min(m6[:], r1[:], 6.0)
                nc.vector.tensor_tensor(h_t[:, h * 2 + dvt, :], y0[:], m6[:],
                                        MULT)

        prev = None  # (h_t, b, yst) pending proj, pipelined one b behind
        xb_next = xb_first
        for b in range(BPC):
            xb = xb_next
            qk_sb = qkio.tile([128, 12, 256], BF16, name=f"qk{b}", tag="qk")
            v_b = [vio.tile([128, DH], BF16, name=f"vb{b}_{tt}", tag=f"vb{tt}",
                            bufs=2) for tt in range(2)]
            h_t = hb.tile([128, 24, 256], BF16, name=f"ht{b}", tag="ht")

            # head-pair sections, software-pipelined: section j runs qk/v GEMM
            # slice j, S matmuls for heads 2j/2j+1, exp for them, and the
            # B-stage (rowsum/O/hardswish) of the PREVIOUS pair, so every PE op
            # has its cross-engine inputs ready.
            for j in range(6):
                qps = ps.tile([128, 512], F32, name=f"qps{b}_{j}", tag="qps",
                              bufs=1)
                for half, cot in ((0, j), (1, 6 + j)):
                    for c in range(6):
                        nc.tensor.matmul(
                            qps[:, half * 256:(half + 1) * 256],
                            wqk_l[c][:, cot * 128:(cot + 1) * 128],
                            xb[:, c, :], start=(c == 0), stop=(c == 5))
                for half, cot in ((0, j), (1, 6 + j)):
                    nc.scalar.activation(
                        qk_sb[:, cot, :], qps[:, half * 256:(half + 1) * 256],
                        mybir.ActivationFunctionType.Identity,
                        bias=bqk_sb[:, cot:cot + 1], scale=1.0)
                for tt in range(2):
                    vps = ps.tile([128, 512], F32, name=f"vps{b}_{tt}_{j}",
                                  tag="vps", bufs=1)
                    for c in range(6):
                        nc.tensor.matmul(
                            vps[:], xb[:, c, tt * 128:(tt + 1) * 128],
                            wv_l[c][:, j * 512:(j + 1) * 512],
                            start=(c == 0), stop=(c == 5))
                    nc.vector.tensor_tensor(
                        v_b[tt][:, j * 512:(j + 1) * 512], vps[:],
                        bvb_sb[:, j * 512:(j + 1) * 512], ADD)
                emit_head_A(b, 2 * j, qk_sb)
                emit_head_A(b, 2 * j + 1, qk_sb)
                emit_exp(b, 2 * j)
                emit_exp(b, 2 * j + 1)
                if j == 2 and b + 1 < BPC:
                    xb_next = load_x(b + 1)
                if j > 0:
                    emit_head_B(b, 2 * (j - 1), v_b, h_t)
                    emit_head_B(b, 2 * (j - 1) + 1, v_b, h_t)
                    if prev is not None:
                        emit_proj_group(prev[0], prev[1], j - 1, prev[2])
            # drain the last pair + proj group of this b
            emit_head_B(b, 10, v_b, h_t)
            emit_head_B(b, 11, v_b, h_t)
            if prev is not None:
                emit_proj_group(prev[0], prev[1], 5, prev[2])

            yst = yio.tile([128, 6, 256], F32, name=f"yst{b}", tag="yst")
            prev = (h_t, b, yst)

        # drain the last b's proj
        for ct in range(6):
            emit_proj_group(prev[0], prev[1], ct, prev[2])


def _build(reps=1, phase="both"):
    nc = bacc.Bacc("TRN2", target_bir_lowering=False, debug=False)
    xbf_t = nc.dram_tensor("xbf_t", [6, 128, T], BF16, kind="ExternalInput")
    wqk_t = nc.dram_tensor("wqk_t", [6, 128, 1536], BF16, kind="ExternalInput")
    wv_t = nc.dram_tensor("wv_t", [6, 128, DH], BF16, kind="ExternalInput")
    wp_t = nc.dram_tensor("wp_t", [24, 128, DIM], BF16, kind="ExternalInput")
    bqk = nc.dram_tensor("bqk", [128, 12], F32, kind="ExternalInput")
    bv = nc.dram_tensor("bv", [DH], BF16, kind="ExternalInput")
    bsc = nc.dram_tensor("bsc", [12, 2, 128, 256], BF16, kind="ExternalInput")
    pg = nc.dram_tensor("pg", [128, 6], F32, kind="ExternalInput")
    pb = nc.dram_tensor("pb", [128, 6], F32, kind="ExternalInput")
    y_t = nc.dram_tensor("y_t", [6, 128, T], F32, kind="ExternalOutput")
    ones_c = nc.dram_tensor("ones_c", [128, 1], F32R, kind="ExternalInput")

    with tile.TileContext(nc) as tc:
        if reps == 1:
            _fused(tc, nc, xbf_t, wqk_t, wv_t, wp_t, bqk, bv, bsc, pg, pb,
                   ones_c, y_t)
        else:
            with tc.For_i(0, reps, 1):
                _fused(tc, nc, xbf_t, wqk_t, wv_t, wp_t, bqk, bv, bsc, pg, pb,
                       ones_c, y_t)
    nc.compile()
    return nc


_NC = None


def _get_nc():
    global _NC
    if _NC is None:
        _NC = _build()
    return _NC


def _prep_host(qkv_w, qkv_gamma, qkv_beta, qkv_mean, qkv_var,
               attn_biases, proj_w, proj_gamma, proj_beta, proj_mean, proj_var,
               bias_idxs):
    f32 = np.float32
    bf16 = ml_dtypes.bfloat16
    qkv_w = np.asarray(qkv_w, f32)
    s = np.asarray(qkv_gamma, f32) / np.sqrt(np.asarray(qkv_var, f32) + EPS)
    Wf = qkv_w * s[:, None]
    bf = np.asarray(qkv_beta, f32) - np.asarray(qkv_mean, f32) * s

    # channel order: cot 0-5 = q blocks (heads 2j,2j+1 per 128), 6-11 = k blocks
    base = np.arange(HEADS, dtype=np.int64) * 384
    q_ch = (base[:, None] + np.arange(64)[None, :]).reshape(-1)          # q rows
    k_ch = (base[:, None] + 64 + np.arange(64)[None, :]).reshape(-1)     # k rows
    v_ch = (base[:, None] + 128 + np.arange(256)[None, :]).reshape(-1)
    qk_ch = np.concatenate([q_ch, k_ch])

    Wqk = Wf[qk_ch].copy()
    bqk_v = bf[qk_ch].copy()
    Wqk[:768] *= SCALE      # fold softmax scale into q
    bqk_v[:768] *= SCALE

    wqk_t = np.ascontiguousarray(Wqk.T).reshape(6, 128, 1536).astype(bf16)
    wv_t = np.ascontiguousarray(Wf[v_ch].T).reshape(6, 128, DH).astype(bf16)
    bqk_np = np.ascontiguousarray(bqk_v.reshape(12, 128).T)
    bv_np = bf[v_ch].astype(bf16)

    proj_w = np.asarray(proj_w, f32)
    sp = np.asarray(proj_gamma, f32) / np.sqrt(np.asarray(proj_var, f32) + EPS)
    bp_v = np.asarray(proj_beta, f32) - np.asarray(proj_mean, f32) * sp
    wp_t = np.ascontiguousarray(proj_w.T).reshape(24, 128, DIM).astype(bf16)
    pg_np = np.ascontiguousarray(sp.reshape(6, 128).T)
    pb_np = np.ascontiguousarray(bp_v.reshape(6, 128).T)

    bias_full = np.asarray(attn_biases, f32)[:, np.asarray(bias_idxs)]  # [h,i,j]
    bsc_np = np.ascontiguousarray(
        bias_full.transpose(0, 2, 1)).reshape(HEADS, 2, 128, 256).astype(bf16)

    return dict(wqk_t=wqk_t, wv_t=wv_t, bqk=bqk_np, bv=bv_np,
                wp_t=wp_t, pg=pg_np, pb=pb_np, bsc=bsc_np,
                ones_c=np.full((128, 1), 6.0, f32))


def kernel(x, qkv_w, qkv_gamma, qkv_beta, qkv_mean, qkv_var,
           attn_biases, proj_w, proj_gamma, proj_beta, proj_mean, proj_var,
           bias_idxs):
    x = np.asarray(x, np.float32)
    shared = _prep_host(qkv_w, qkv_gamma, qkv_beta, qkv_mean, qkv_var,
                        attn_biases, proj_w, proj_gamma, proj_beta,
                        proj_mean, proj_var, bias_idxs)
    in_maps = []
    for ci in range(NCORES):
        xc = x[ci * BPC:(ci + 1) * BPC].reshape(T, DIM)
        x_tc = np.ascontiguousarray(xc.T).reshape(6, 128, T).astype(
            ml_dtypes.bfloat16)
        m = dict(shared)
        m["xbf_t"] = x_tc
        in_maps.append(m)

    nc = _get_nc()
    res = run_bass_kernel_spmd(nc, in_maps, core_ids=list(range(NCORES)))

    out = np.empty((B, SEQ, DIM), np.float32)
    for ci in range(NCORES):
        yt = np.asarray(res.results[ci]["y_t"]).reshape(DIM, T)
        out[ci * BPC:(ci + 1) * BPC] = yt.T.reshape(BPC, SEQ, DIM)
    return out
